# revision 13
# baseline (speedup 1.0000x reference)
"""Trainium2 Bass kernel for nn_CELoss_4896262717859.

For each query column c = idx_node[k] of a sparse adjacency matrix (diagonal
zeroed), a cross-entropy-style loss over the "lower" (r < c) and "upper"
(r >= c) neighbor sets:

    contrib_side(c) = [cnt>0 and poscnt==1] * (lse - poslogit) / cnt

Strategy (v5):
  * Host gathers only the UNIQUE query columns (~3.2K of 8192, idx_node has
    ~21% duplicates) -> fp8 (values 0/1, exact). Duplicates are applied as
    multiplicities in the host combine. ~3.4MB/core vs 33.5MB int32 baseline.
  * Device: per-core column slab, 32 fp8 DoubleRow matmuls — each covers a
    PAIR of 128-row tiles at 0.5 PE cycles/column — producing per-pair stats
    [6, slab] = {ones, pos, pl_hi, pl_lo*SC, e_hi, e_lo*SC}. No masking, no
    casts; psum->sbuf copies alternate DVE/ACT; one in-order full-bandwidth
    DMA stream on the sync queue.
  * Host combine: the L/U split at row idx_node[k] = prefix sum over the 32
    per-pair stats + an exact 256-row partial for the boundary pair. O(K).
"""

import numpy as np
import ml_dtypes

N = 8192
K = 4096
NCORES = 8
P = 128                   # partition / tile edge
NT = N // P               # 64 row tiles
PAIRS = NT // 2           # 32 row-tile pairs (DoubleRow granularity)
ROWS_PER_PAIR = 2 * P     # 256
NW = 6                    # stat components per column
SC = 32.0                 # scale for the *_lo fp8 channels
NWP = 16                  # weight inner-dim padding: dual-fp8 ldweights
                          # requires a 16-byte-aligned k-plane stride
# row tiles per input DMA: big chunks amortize DMA overheads and keep
# descriptor lines >= 4KB (full HBM bandwidth); small trailing chunks keep
# the end-of-stream matmul/sem-prop tail short
CHUNKS = [16, 16, 16, 8, 4, 2, 2]

FP8 = ml_dtypes.float8_e4m3   # == mybir.dt.np(mybir.dt.float8e4); max 240

_BASS_CACHE = {}


def _build_bass(slab):
    import concourse.tile as tile
    import concourse.mybir as mybir
    from concourse import bacc

    nc = bacc.Bacc("TRN2")
    # g{c}[p, u, n] = adjacency row 128*(tile_base_c+u)+p, slab column n
    # (0/1 fp8). Chunk-major: each chunk DMA reads one fully contiguous DRAM
    # block (strided reads at 32KB pitch only reach ~260GB/s vs ~356).
    gs = [
        nc.dram_tensor(
            f"g{c}", [P, t, slab], mybir.dt.float8e4, kind="ExternalInput"
        )
        for c, t in enumerate(CHUNKS)
    ]
    # w[p, j, m] = weight component m for row 128*j+p
    w = nc.dram_tensor("w", [P, NT, NWP], mybir.dt.float8e4, kind="ExternalInput")
    stats = nc.dram_tensor(
        "stats", [NW, PAIRS * slab], mybir.dt.float32, kind="ExternalOutput"
    )

    with tile.TileContext(nc) as tc:
        with (
            tc.tile_pool(name="singles", bufs=1) as singles,
            tc.tile_pool(name="io", bufs=1) as io_pool,
            # 8 single-pair psum tiles: copies may lag matmuls by up to 8
            # pairs, so the copy chain never throttles the matmul stream
            tc.tile_pool(name="psum", bufs=8, space="PSUM") as psum_pool,
        ):
            # weights on the ACT HW queue: sync's queue stays a pure,
            # in-order, full-bandwidth g stream
            wsb = singles.tile([P, NT, NWP], mybir.dt.float8e4)
            nc.scalar.dma_start(out=wsb, in_=w[:, :, :])
            chunks = []   # (tile, tile_base) per chunk
            base = 0
            for c, tcnt in enumerate(CHUNKS):
                t = io_pool.tile(
                    [P, tcnt, slab], mybir.dt.float8e4, tag=f"g{c}", name=f"gt{c}"
                )
                nc.sync.dma_start(out=t, in_=gs[c][:, :, :])
                chunks.append((t, base))
                base += tcnt

            out_sb = singles.tile([NW, PAIRS * slab], mybir.dt.float32)

            cidx = 0
            for q in range(PAIRS):
                acc = psum_pool.tile(
                    [NW, slab], mybir.dt.float32, tag="acc", name=f"acc{q}"
                )
                while 2 * q >= chunks[cidx][1] + CHUNKS[cidx]:
                    cidx += 1
                ch, cbase = chunks[cidx]
                off = 2 * q - cbase
                # DoubleRow: lhsT [128, 2, 6], rhs [128, 2, slab] -> [6, slab]
                # = sum over the two 128-row tiles at 0.5 cycles/column.
                nc.tensor.matmul(
                    acc,
                    wsb[:, 2 * q : 2 * q + 2, 0:NW],
                    ch[:, off : off + 2, :],
                    start=True,
                    stop=True,
                    perf_mode=mybir.MatmulPerfMode.DoubleRow,
                )
                # psum -> sbuf staging alternates DVE/ACT so neither engine's
                # copy chain paces the matmul stream
                dst = out_sb[:, q * slab : (q + 1) * slab]
                if q % 2 == 0:
                    nc.vector.tensor_copy(dst, acc)
                else:
                    nc.scalar.copy(dst, acc)
                # stats out in 2 groups on sync's queue (idle once the g
                # issues are done) so ACT never stalls its copy chain
                if (q + 1) % (PAIRS // 2) == 0:
                    s = (q + 1 - PAIRS // 2) * slab
                    e = (q + 1) * slab
                    nc.sync.dma_start(out=stats[:, s:e], in_=out_sb[:, s:e])

    nc.compile()
    return nc


def _host_prep(outputs, targets):
    """Quantized weight table [128, 64, 16] fp8 + exact f64 weights [8192, 4]."""
    out = np.asarray(outputs, np.float64).reshape(-1)
    pos = (np.asarray(targets).reshape(-1) != 0).astype(np.float64)
    # shift exp into fp8 range only if needed (max normal 240 -> ln 240 = 5.48)
    b_shift = max(0.0, float(out.max()) - 4.5)
    pl = pos * out
    ev = np.exp(out - b_shift)

    def split(v):
        hi = v.astype(FP8)
        lo = ((v - hi.astype(np.float64)) * SC).astype(FP8)
        return hi, lo

    pl_hi, pl_lo = split(pl)
    e_hi, e_lo = split(ev)
    wq = np.stack(
        [np.ones(N, FP8), pos.astype(FP8), pl_hi, pl_lo, e_hi, e_lo], axis=1
    )  # [N, 6] fp8
    wtrue = np.stack([np.ones(N), pos, pl, ev], axis=1)  # [N, 4] f64
    wpad = np.zeros((N, NWP), FP8)
    wpad[:, :NW] = wq
    wmat = np.ascontiguousarray(wpad.reshape(NT, P, NWP).transpose(1, 0, 2))
    return wmat, wtrue, b_shift


def _build_shard(Gpad, core, slab):
    """Per-core dict of chunk arrays [128, t, slab] fp8 from [N, slab]."""
    cols = Gpad[:, core * slab : (core + 1) * slab].astype(FP8)
    tiles = cols.reshape(NT, P, slab)
    shard = {}
    base = 0
    for c, t in enumerate(CHUNKS):
        arr = tiles[base : base + t].transpose(1, 0, 2)
        shard[f"g{c}"] = np.ascontiguousarray(arr)
        base += t
    return shard


def _prep(outputs, targets, node_adj, idx_node):
    wmat, wtrue, b_shift = _host_prep(outputs, targets)
    idx = np.asarray(idx_node).reshape(-1).astype(np.int64)
    uniq, counts = np.unique(idx, return_counts=True)
    ku = uniq.size
    slab = -(-ku // (NCORES * 16)) * 16      # per-core columns, multiple of 16
    kpad = slab * NCORES
    # G[r, k] = node_adj[r, uniq[k]] != 0, diag zeroed; zero-padded columns
    G = np.zeros((N, kpad), np.uint8)
    G[:, :ku] = np.asarray(node_adj)[:, uniq] != 0
    G[uniq, np.arange(ku)] = 0               # node_adj[diag] = 0
    in_maps = [dict(_build_shard(G, d, slab), w=wmat) for d in range(NCORES)]
    ctx = {
        "uniq": uniq, "counts": counts, "G": G, "wtrue": wtrue,
        "b_shift": b_shift, "slab": slab, "ku": ku,
    }
    return in_maps, ctx


def _combine(stats_list, ctx):
    """stats_list: per-core [6, PAIRS*slab] f32 -> scalar loss (f64 math)."""
    uniq, counts, G, wtrue, b_shift, slab, ku = (
        ctx["uniq"], ctx["counts"], ctx["G"], ctx["wtrue"],
        ctx["b_shift"], ctx["slab"], ctx["ku"],
    )
    kpad = slab * NCORES
    # Sg[q, m, k]: per-pair stats for all padded columns
    Sg = np.empty((PAIRS, NW, kpad), np.float64)
    for c, s in enumerate(stats_list):
        Sg[:, :, c * slab : (c + 1) * slab] = (
            np.asarray(s, np.float64).reshape(NW, PAIRS, slab).transpose(1, 0, 2)
        )
    Sg = Sg[:, :, :ku]
    C = np.concatenate(
        [np.zeros((1, NW, ku)), np.cumsum(Sg, axis=0)], axis=0
    )  # [PAIRS+1, 6, ku]

    kk = np.arange(ku)
    qk = (uniq // ROWS_PER_PAIR).astype(np.int64)
    L_raw = C[qk, :, kk]                        # [ku, 6] full pairs below split
    U_raw = C[PAIRS, :, kk] - C[qk + 1, :, kk]  # full pairs above split

    # exact f64 partial for the boundary pair (256 rows containing uniq[k])
    rows = qk[None, :] * ROWS_PER_PAIR + np.arange(ROWS_PER_PAIR)[:, None]
    gpair = G[rows, kk[None, :]].astype(np.float64)
    low = (rows < uniq[None, :]).astype(np.float64)
    glo = gpair * low
    ghi = gpair - glo

    def side(raw, gm):
        cnt = raw[:, 0] + gm.sum(axis=0)
        poscnt = raw[:, 1] + (gm * wtrue[rows, 1]).sum(axis=0)
        pl = raw[:, 2] + raw[:, 3] / SC + (gm * wtrue[rows, 2]).sum(axis=0)
        ev = raw[:, 4] + raw[:, 5] / SC + (gm * wtrue[rows, 3]).sum(axis=0)
        valid = (cnt > 0.5) & (np.abs(poscnt - 1.0) < 0.25)
        lse = np.log(np.where(valid, np.maximum(ev, 1e-300), 1.0)) + b_shift
        return np.where(valid, (lse - pl) / np.maximum(cnt, 1.0), 0.0)

    contrib = side(L_raw, glo) + side(U_raw, ghi)
    return np.array((contrib * counts).sum(), dtype=np.float32)


def _ensure_axon_hooks_stub():
    """bass_utils imports antenv.axon_hooks when tracing is requested via
    env; the module is absent on some images. Provide a no-op stub so the
    import never crashes (hook=None -> bass_utils skips tracing)."""
    import sys
    import types

    try:
        import antenv.axon_hooks  # noqa: F401
    except ImportError:
        mod = types.ModuleType("antenv.axon_hooks")
        state = {"hook": None}
        mod.set_axon_ntff_profile_hook = lambda h: state.__setitem__("hook", h)
        mod.get_axon_ntff_profile_hook = lambda: state["hook"]
        sys.modules["antenv.axon_hooks"] = mod


def _device_stats(in_maps, slab):
    _ensure_axon_hooks_stub()
    from concourse.bass_utils import run_bass_kernel_spmd

    if slab not in _BASS_CACHE:
        _BASS_CACHE[slab] = _build_bass(slab)
    last_exc = None
    for attempt in range(4):
        try:
            res = run_bass_kernel_spmd(
                _BASS_CACHE[slab], in_maps, core_ids=list(range(NCORES))
            )
            return [r["stats"] for r in res.results]
        except Exception as e:  # transient NRT/accelerator hiccups
            last_exc = e
            try:
                # a fresh PJRT client usually recovers a transiently
                # "unrecoverable" accelerator; mirrors a process restart
                import jax
                import jax.extend.backend as _jeb

                jax.clear_caches()
                _jeb.clear_backends()
            except Exception:
                pass
            import time

            time.sleep(2.0 * (attempt + 1))
    raise last_exc


def _sim_stats(in_maps, slab):
    """Numpy emulation of the device kernel (same inputs), for validation."""
    outs = []
    for m in in_maps:
        gm = np.concatenate(
            [m[f"g{c}"].astype(np.float32).transpose(1, 0, 2) for c in range(len(CHUNKS))],
            axis=0,
        ).transpose(1, 0, 2)  # [128, 64, slab]
        wm = m["w"].astype(np.float32)[:, :, :NW]  # [128, 64, 6]
        acc = np.zeros((NW, PAIRS, slab), np.float32)
        for q in range(PAIRS):
            for j in (2 * q, 2 * q + 1):
                acc[:, q, :] += wm[:, j, :].T @ gm[:, j, :]
        outs.append(acc.reshape(NW, PAIRS * slab))
    return outs


def kernel(outputs, targets, node_adj, idx_node, _simulate=False):
    in_maps, ctx = _prep(outputs, targets, node_adj, idx_node)
    slab = ctx["slab"]
    stats = _sim_stats(in_maps, slab) if _simulate else _device_stats(in_maps, slab)
    return _combine(stats, ctx)


# revision 14
# speedup vs baseline: 1.1479x; 1.1479x over previous
"""Trainium2 Bass kernel for nn_CELoss_4896262717859.

For each query column c = idx_node[k] of a sparse adjacency matrix (diagonal
zeroed), a cross-entropy-style loss over the "lower" (r < c) and "upper"
(r >= c) neighbor sets:

    contrib_side(c) = [cnt>0 and poscnt==1] * (lse - poslogit) / cnt

Strategy (v5):
  * Host gathers only the UNIQUE query columns (~3.2K of 8192, idx_node has
    ~21% duplicates) -> fp8 (values 0/1, exact). Duplicates are applied as
    multiplicities in the host combine. ~3.4MB/core vs 33.5MB int32 baseline.
  * Device: per-core column slab, 32 fp8 DoubleRow matmuls — each covers a
    PAIR of 128-row tiles at 0.5 PE cycles/column — producing per-pair stats
    [6, slab] = {ones, pos, pl_hi, pl_lo*SC, e_hi, e_lo*SC}. No masking, no
    casts; psum->sbuf copies alternate DVE/ACT; one in-order full-bandwidth
    DMA stream on the sync queue.
  * Host combine: the L/U split at row idx_node[k] = prefix sum over the 32
    per-pair stats + an exact 256-row partial for the boundary pair. O(K).
"""

import numpy as np
import ml_dtypes

N = 8192
K = 4096
NCORES = 8
P = 128                   # partition / tile edge
NT = N // P               # 64 row tiles
PAIRS = NT // 2           # 32 row-tile pairs (DoubleRow granularity)
GP = 4                    # pairs accumulated per psum group (matmul chaining)
GROUPS = PAIRS // GP      # 8 device output rows of stats
ROWS_PER_GROUP = GP * 2 * P   # 1024
NW = 6                    # stat components per column
SC = 32.0                 # scale for the *_lo fp8 channels
NWP = 16                  # weight inner-dim padding: dual-fp8 ldweights
                          # requires a 16-byte-aligned k-plane stride
# row tiles per input DMA: big chunks amortize DMA overheads and keep
# descriptor lines >= 4KB (full HBM bandwidth); small trailing chunks keep
# the end-of-stream matmul/sem-prop tail short
CHUNKS = [16, 16, 16, 8, 4, 2, 2]

FP8 = ml_dtypes.float8_e4m3   # == mybir.dt.np(mybir.dt.float8e4); max 240

_BASS_CACHE = {}


def _build_bass(slab):
    import concourse.tile as tile
    import concourse.mybir as mybir
    from concourse import bacc

    nc = bacc.Bacc("TRN2")
    # g{c}[p, u, n] = adjacency row 128*(tile_base_c+u)+p, slab column n
    # (0/1 fp8). Chunk-major: each chunk DMA reads one fully contiguous DRAM
    # block (strided reads at 32KB pitch only reach ~260GB/s vs ~356).
    gs = [
        nc.dram_tensor(
            f"g{c}", [P, t, slab], mybir.dt.float8e4, kind="ExternalInput"
        )
        for c, t in enumerate(CHUNKS)
    ]
    # w[p, j, m] = weight component m for row 128*j+p
    w = nc.dram_tensor("w", [P, NT, NWP], mybir.dt.float8e4, kind="ExternalInput")
    stats = nc.dram_tensor(
        "stats", [NW, GROUPS * slab], mybir.dt.float32, kind="ExternalOutput"
    )

    with tile.TileContext(nc) as tc:
        with (
            tc.tile_pool(name="singles", bufs=1) as singles,
            tc.tile_pool(name="io", bufs=1) as io_pool,
            # 8 single-pair psum tiles: copies may lag matmuls by up to 8
            # pairs, so the copy chain never throttles the matmul stream
            tc.tile_pool(name="psum", bufs=8, space="PSUM") as psum_pool,
        ):
            # weights on the ACT HW queue: sync's queue stays a pure,
            # in-order, full-bandwidth g stream
            wsb = singles.tile([P, NT, NWP], mybir.dt.float8e4)
            nc.scalar.dma_start(out=wsb, in_=w[:, :, :])
            chunks = []   # (tile, tile_base) per chunk
            base = 0
            for c, tcnt in enumerate(CHUNKS):
                t = io_pool.tile(
                    [P, tcnt, slab], mybir.dt.float8e4, tag=f"g{c}", name=f"gt{c}"
                )
                nc.sync.dma_start(out=t, in_=gs[c][:, :, :])
                chunks.append((t, base))
                base += tcnt

            out_sb = singles.tile([NW, GROUPS * slab], mybir.dt.float32)

            cidx = 0
            acc = None
            for q in range(PAIRS):
                gi, within = q // GP, q % GP
                if within == 0:
                    acc = psum_pool.tile(
                        [NW, slab], mybir.dt.float32, tag="acc", name=f"acc{gi}"
                    )
                while 2 * q >= chunks[cidx][1] + CHUNKS[cidx]:
                    cidx += 1
                ch, cbase = chunks[cidx]
                off = 2 * q - cbase
                # DoubleRow: lhsT [128, 2, 6], rhs [128, 2, slab] -> [6, slab]
                # = sum over the two 128-row tiles at 0.5 cycles/column.
                # GP pairs ACCUMULATE in psum (start/stop chaining), so only
                # GROUPS copies + a 1024-row host boundary partial are needed
                # -> the copy chain never paces the stream.
                nc.tensor.matmul(
                    acc,
                    wsb[:, 2 * q : 2 * q + 2, 0:NW],
                    ch[:, off : off + 2, :],
                    start=(within == 0),
                    stop=(within == GP - 1),
                    perf_mode=mybir.MatmulPerfMode.DoubleRow,
                )
                if within == GP - 1:
                    dst = out_sb[:, gi * slab : (gi + 1) * slab]
                    if gi % 2 == 0:
                        nc.vector.tensor_copy(dst, acc)
                    else:
                        nc.scalar.copy(dst, acc)
                    # stats out in 2 halves on sync's queue (idle once the
                    # g issues are done)
                    if gi % (GROUPS // 2) == GROUPS // 2 - 1:
                        s = (gi - GROUPS // 2 + 1) * slab
                        e = (gi + 1) * slab
                        nc.sync.dma_start(out=stats[:, s:e], in_=out_sb[:, s:e])

    nc.compile()
    return nc


def _host_prep(outputs, targets):
    """Quantized weight table [128, 64, 16] fp8 + exact f64 weights [8192, 4]."""
    out = np.asarray(outputs, np.float64).reshape(-1)
    pos = (np.asarray(targets).reshape(-1) != 0).astype(np.float64)
    # shift exp into fp8 range only if needed (max normal 240 -> ln 240 = 5.48)
    b_shift = max(0.0, float(out.max()) - 4.5)
    pl = pos * out
    ev = np.exp(out - b_shift)

    def split(v):
        hi = v.astype(FP8)
        lo = ((v - hi.astype(np.float64)) * SC).astype(FP8)
        return hi, lo

    pl_hi, pl_lo = split(pl)
    e_hi, e_lo = split(ev)
    wq = np.stack(
        [np.ones(N, FP8), pos.astype(FP8), pl_hi, pl_lo, e_hi, e_lo], axis=1
    )  # [N, 6] fp8
    wtrue = np.stack([np.ones(N), pos, pl, ev], axis=1)  # [N, 4] f64
    wpad = np.zeros((N, NWP), FP8)
    wpad[:, :NW] = wq
    wmat = np.ascontiguousarray(wpad.reshape(NT, P, NWP).transpose(1, 0, 2))
    return wmat, wtrue, b_shift


def _build_shard(Gpad, core, slab):
    """Per-core dict of chunk arrays [128, t, slab] fp8 from [N, slab]."""
    cols = Gpad[:, core * slab : (core + 1) * slab].astype(FP8)
    tiles = cols.reshape(NT, P, slab)
    shard = {}
    base = 0
    for c, t in enumerate(CHUNKS):
        arr = tiles[base : base + t].transpose(1, 0, 2)
        shard[f"g{c}"] = np.ascontiguousarray(arr)
        base += t
    return shard


def _prep(outputs, targets, node_adj, idx_node):
    wmat, wtrue, b_shift = _host_prep(outputs, targets)
    idx = np.asarray(idx_node).reshape(-1).astype(np.int64)
    uniq, counts = np.unique(idx, return_counts=True)
    ku = uniq.size
    slab = -(-ku // (NCORES * 16)) * 16      # per-core columns, multiple of 16
    kpad = slab * NCORES
    # G[r, k] = node_adj[r, uniq[k]] != 0, diag zeroed; zero-padded columns
    G = np.zeros((N, kpad), np.uint8)
    G[:, :ku] = np.asarray(node_adj)[:, uniq] != 0
    G[uniq, np.arange(ku)] = 0               # node_adj[diag] = 0
    in_maps = [dict(_build_shard(G, d, slab), w=wmat) for d in range(NCORES)]
    ctx = {
        "uniq": uniq, "counts": counts, "G": G, "wtrue": wtrue,
        "b_shift": b_shift, "slab": slab, "ku": ku,
    }
    return in_maps, ctx


def _combine(stats_list, ctx):
    """stats_list: per-core [6, PAIRS*slab] f32 -> scalar loss (f64 math)."""
    uniq, counts, G, wtrue, b_shift, slab, ku = (
        ctx["uniq"], ctx["counts"], ctx["G"], ctx["wtrue"],
        ctx["b_shift"], ctx["slab"], ctx["ku"],
    )
    kpad = slab * NCORES
    # Sg[g, m, k]: per-group stats for all padded columns
    Sg = np.empty((GROUPS, NW, kpad), np.float64)
    for c, s in enumerate(stats_list):
        Sg[:, :, c * slab : (c + 1) * slab] = (
            np.asarray(s, np.float64).reshape(NW, GROUPS, slab).transpose(1, 0, 2)
        )
    Sg = Sg[:, :, :ku]
    C = np.concatenate(
        [np.zeros((1, NW, ku)), np.cumsum(Sg, axis=0)], axis=0
    )  # [GROUPS+1, 6, ku]

    kk = np.arange(ku)
    qk = (uniq // ROWS_PER_GROUP).astype(np.int64)
    L_raw = C[qk, :, kk]                         # [ku, 6] full groups below
    U_raw = C[GROUPS, :, kk] - C[qk + 1, :, kk]  # full groups above

    # exact f64 partial for the boundary group (1024 rows containing uniq[k])
    rows = qk[None, :] * ROWS_PER_GROUP + np.arange(ROWS_PER_GROUP)[:, None]
    gpair = G[rows, kk[None, :]].astype(np.float64)
    low = (rows < uniq[None, :]).astype(np.float64)
    glo = gpair * low
    ghi = gpair - glo

    def side(raw, gm):
        cnt = raw[:, 0] + gm.sum(axis=0)
        poscnt = raw[:, 1] + (gm * wtrue[rows, 1]).sum(axis=0)
        pl = raw[:, 2] + raw[:, 3] / SC + (gm * wtrue[rows, 2]).sum(axis=0)
        ev = raw[:, 4] + raw[:, 5] / SC + (gm * wtrue[rows, 3]).sum(axis=0)
        valid = (cnt > 0.5) & (np.abs(poscnt - 1.0) < 0.25)
        lse = np.log(np.where(valid, np.maximum(ev, 1e-300), 1.0)) + b_shift
        return np.where(valid, (lse - pl) / np.maximum(cnt, 1.0), 0.0)

    contrib = side(L_raw, glo) + side(U_raw, ghi)
    return np.array((contrib * counts).sum(), dtype=np.float32)


def _ensure_axon_hooks_stub():
    """bass_utils imports antenv.axon_hooks when tracing is requested via
    env; the module is absent on some images. Provide a no-op stub so the
    import never crashes (hook=None -> bass_utils skips tracing)."""
    import sys
    import types

    try:
        import antenv.axon_hooks  # noqa: F401
    except ImportError:
        mod = types.ModuleType("antenv.axon_hooks")
        state = {"hook": None}
        mod.set_axon_ntff_profile_hook = lambda h: state.__setitem__("hook", h)
        mod.get_axon_ntff_profile_hook = lambda: state["hook"]
        sys.modules["antenv.axon_hooks"] = mod


def _device_stats(in_maps, slab):
    _ensure_axon_hooks_stub()
    from concourse.bass_utils import run_bass_kernel_spmd

    if slab not in _BASS_CACHE:
        _BASS_CACHE[slab] = _build_bass(slab)
    last_exc = None
    for attempt in range(4):
        try:
            res = run_bass_kernel_spmd(
                _BASS_CACHE[slab], in_maps, core_ids=list(range(NCORES))
            )
            return [r["stats"] for r in res.results]
        except Exception as e:  # transient NRT/accelerator hiccups
            last_exc = e
            try:
                # a fresh PJRT client usually recovers a transiently
                # "unrecoverable" accelerator; mirrors a process restart
                import jax
                import jax.extend.backend as _jeb

                jax.clear_caches()
                _jeb.clear_backends()
            except Exception:
                pass
            import time

            time.sleep(2.0 * (attempt + 1))
    raise last_exc


def _sim_stats(in_maps, slab):
    """Numpy emulation of the device kernel (same inputs), for validation."""
    outs = []
    for m in in_maps:
        gm = np.concatenate(
            [m[f"g{c}"].astype(np.float32).transpose(1, 0, 2) for c in range(len(CHUNKS))],
            axis=0,
        ).transpose(1, 0, 2)  # [128, 64, slab]
        wm = m["w"].astype(np.float32)[:, :, :NW]  # [128, 64, 6]
        acc = np.zeros((NW, GROUPS, slab), np.float32)
        for q in range(PAIRS):
            for j in (2 * q, 2 * q + 1):
                acc[:, q // GP, :] += wm[:, j, :].T @ gm[:, j, :]
        outs.append(acc.reshape(NW, GROUPS * slab))
    return outs


def kernel(outputs, targets, node_adj, idx_node, _simulate=False):
    in_maps, ctx = _prep(outputs, targets, node_adj, idx_node)
    slab = ctx["slab"]
    stats = _sim_stats(in_maps, slab) if _simulate else _device_stats(in_maps, slab)
    return _combine(stats, ctx)
